# revision 1
# baseline (speedup 1.0000x reference)
"""BackProjNet kernel for 8 Trainium2 NeuronCores.

Strategy (sharding_hint: shard voxel axis):
  - 65536 voxels split across 8 cores (8192 voxels / 8.39M elements each).
  - Per element we need x[idx]*w summed per voxel (segments of 1024).
  - The sinogram is viewed as 5888 rows x 64 f32 (256B rows). For each
    element, dma_gather fetches the 256B row containing x[idx] from HBM
    into SBUF; a DVE one-hot select (off = idx & 63) picks the target
    value, multiplies by w, and reduces; the PE sums partitions per voxel.
  - Scale/bias/flip epilogue on host (O(output) work only).
"""
import sys

sys.path.insert(0, "/opt/trn_rl_repo")

import numpy as np

_P = 128
_ROWS = 5888
_ES = 64                 # f32 per gathered row (256B)
_N = 8192                # elements per tile
_K = _N // _P            # 64 free slots per partition per tile
_NCORES = 8
_VOX = 65536
_VPC = _VOX // _NCORES   # 8192 voxels per core
_EPC = _VPC * 1024       # 8388608 elements per core
_T = _EPC // _N          # 1024 tiles per core
_VPT = _N // 1024        # 8 voxels per tile
_SCALE = np.float32(2.0 * np.pi / (2.0 * 512 * 2))

_cache = {}


def _patch_tile_drain():
    import bass_rust
    import concourse.tile as tile_mod
    from concourse.vector_clock import ScopedClock

    if getattr(tile_mod.TileContext, "_drain_waits_split", False):
        return

    def _drain_and_barrier(self, tick_clock, wait_clock):
        nc = self.nc
        probe = nc.sync.nop()
        wait_clock.add_sem_waits(
            probe.ins, ScopedClock({None: tick_clock.global_clock})
        )
        si = probe.ins.sync_info
        waits = list(si.on_wait) if si is not None else []
        if len(waits) > 1:
            si.on_wait = [waits[0]]
            for w in waits[1:]:
                extra = nc.sync.nop()
                if extra.ins.sync_info is None:
                    extra.ins.sync_info = bass_rust.SyncInfo(
                        on_wait=[w], on_update=[]
                    )
                else:
                    extra.ins.sync_info.on_wait = [w]
        nc.sync.drain()
        nc.all_engine_barrier()
        assert self.sems is not None
        popped = nc._tile_sem_poison_stack.pop()
        assert popped is self._sem_poison
        nc.clear_and_free_semaphores(list(self.sems.allocated().values()))
        nc.all_engine_barrier()

    tile_mod.TileContext._drain_and_barrier = _drain_and_barrier
    tile_mod.TileContext._drain_waits_split = True


def _split_multi_waits(nc, max_waits=1):
    from concourse import mybir

    ctr = 0
    for f in nc.m.functions:
        for bb in f.blocks:
            insts = list(bb.instructions)
            changed = False
            out = []
            for inst in insts:
                si = getattr(inst, "sync_info", None)
                if si is not None and len(si.on_wait) > max_waits:
                    waits = list(si.on_wait)
                    for w in waits[max_waits:]:
                        ctr += 1
                        nop = mybir.InstNoOp(
                            name=f"wsplit-{ctr}",
                            sync_info=mybir.SyncInfo(on_wait=[w], on_update=[]),
                            bass_nofuse=True,
                            engine=inst.engine,
                        )
                        out.append(nop)
                    si.on_wait = waits[:max_waits]
                    changed = True
                out.append(inst)
            if changed:
                try:
                    bb.instructions = out
                except Exception:
                    bb.clear_instructions()
                    for i in out:
                        bb.add_instruction(i)
    return ctr


def _build_nc():
    from contextlib import ExitStack

    from concourse import bass, library_config, mybir
    from concourse.tile import TileContext

    nc = bass.Bass(
        "TRN2",
        num_devices=_NCORES,
        dynamic_dma_scratch_size=65536,
    )
    xrows = nc.dram_tensor("xrows", [_ROWS, _ES], mybir.dt.float32,
                           kind="ExternalInput")
    rows = nc.dram_tensor("rows", [_T, 32, _N // 16], mybir.dt.int16,
                          kind="ExternalInput")
    offw = nc.dram_tensor("offw", [_T, _P, 2 * _K], mybir.dt.float32,
                          kind="ExternalInput")
    iota = nc.dram_tensor("iota", [_P, _ES], mybir.dt.float32,
                          kind="ExternalInput")
    vout = nc.dram_tensor("vout", [1, _VPC], mybir.dt.float32,
                          kind="ExternalOutput")

    with TileContext(nc) as tc, ExitStack() as ctx:
        nc.gpsimd.load_library(library_config.mlp)
        nreg = nc.gpsimd.to_reg(_N)

        cpool = ctx.enter_context(tc.tile_pool(name="const", bufs=1))
        ipool = ctx.enter_context(tc.tile_pool(name="idx", bufs=3))
        spool = ctx.enter_context(tc.tile_pool(name="stream", bufs=3))
        gpool = ctx.enter_context(tc.tile_pool(name="g", bufs=3))
        epool = ctx.enter_context(tc.tile_pool(name="eq", bufs=3))
        rpool = ctx.enter_context(tc.tile_pool(name="red", bufs=3))
        opool = ctx.enter_context(tc.tile_pool(name="out", bufs=1))
        pspool = ctx.enter_context(
            tc.tile_pool(name="ps", bufs=2, space="PSUM"))

        tio = cpool.tile([_P, _ES], mybir.dt.float32)
        nc.sync.dma_start(tio[:], iota[:, :])
        ones = cpool.tile([_P, 1], mybir.dt.float32)
        nc.vector.memset(ones[:], 1.0)
        voxrow = opool.tile([1, _VPC], mybir.dt.float32)

        for t in range(_T):
            it = ipool.tile([_P, _N // 16], mybir.dt.int16, tag="it")
            nc.sync.dma_start(it[0:32, :], rows[t, :, :])
            st = spool.tile([_P, 2 * _K], mybir.dt.float32, tag="st")
            nc.sync.dma_start(st[:], offw[t, :, :])

            g = gpool.tile([_P, _K * _ES], mybir.dt.float32, tag="g")
            g3 = bass.AP(g.tensor, g.offset,
                         [g.ap[0], [_ES, _K], [1, _ES]])
            nc.gpsimd.dma_gather(
                g3, xrows[:, :], it[:], num_idxs=_N, num_idxs_reg=nreg,
                elem_size=_ES, single_packet=False)

            # EQ[p, k, o] = (off[p, k] == o)
            eq = epool.tile([_P, _K * _ES], mybir.dt.float32, tag="eq")
            eq3 = bass.AP(eq.tensor, eq.offset,
                          [eq.ap[0], [_ES, _K], [1, _ES]])
            off_sl = st[:, 0:_K]
            off_b = bass.AP(off_sl.tensor, off_sl.offset,
                            [off_sl.ap[0], [1, _K], [0, _ES]])
            io_b = bass.AP(tio.tensor, tio.offset,
                           [tio.ap[0], [0, _K], [1, _ES]])
            nc.vector.tensor_tensor(eq3, off_b, io_b,
                                    mybir.AluOpType.is_equal)
            # EQ *= w (broadcast over o)
            w_sl = st[:, _K:2 * _K]
            w_b = bass.AP(w_sl.tensor, w_sl.offset,
                          [w_sl.ap[0], [1, _K], [0, _ES]])
            nc.vector.tensor_tensor(eq3, eq3, w_b, mybir.AluOpType.mult)
            # EQ *= gathered
            nc.vector.tensor_tensor(eq3, eq3, g3, mybir.AluOpType.mult)
            # reduce over o: [P, K]
            r1 = rpool.tile([_P, _K], mybir.dt.float32, tag="r1")
            nc.vector.tensor_reduce(
                bass.AP(r1.tensor, r1.offset, [r1.ap[0], [1, _K], [1, 1]]),
                eq3, mybir.AxisListType.X, mybir.AluOpType.add)
            # reduce k-groups of 8: [P, VPT]
            r2 = rpool.tile([_P, _VPT], mybir.dt.float32, tag="r2")
            r1v = bass.AP(r1.tensor, r1.offset,
                          [r1.ap[0], [_K // _VPT, _VPT], [1, _K // _VPT]])
            r2v = bass.AP(r2.tensor, r2.offset,
                          [r2.ap[0], [1, _VPT], [1, 1]])
            nc.vector.tensor_reduce(r2v, r1v, mybir.AxisListType.X,
                                    mybir.AluOpType.add)
            # partition sum via PE: [1, VPT]
            ps = pspool.tile([1, _VPT], mybir.dt.float32, tag="ps")
            nc.tensor.matmul(ps[:], ones[:], r2[:], start=True, stop=True)
            nc.vector.tensor_copy(voxrow[0:1, t * _VPT:(t + 1) * _VPT],
                                  ps[:])

        nc.sync.dma_start(vout[:, :], voxrow[:])
    return nc


def _get_kernel():
    if "k" in _cache:
        return _cache["k"]
    _patch_tile_drain()
    from concourse import mybir
    from concourse.bass2jax import install_neuronx_cc_hook, partition_id_tensor, _bass_exec_p
    import jax
    from jax.sharding import Mesh, PartitionSpec
    from jax.experimental.shard_map import shard_map

    install_neuronx_cc_hook()
    nc = _build_nc()
    mybir.codegen_inst_isa_subclasses(nc)
    _split_multi_waits(nc)

    partition_name = (nc.partition_id_tensor.name
                      if nc.partition_id_tensor else None)
    in_names, out_names, out_avals = [], [], []
    for alloc in nc.m.functions[0].allocations:
        if not isinstance(alloc, mybir.MemoryLocationSet):
            continue
        name = alloc.memorylocations[0].name
        if alloc.kind == "ExternalInput":
            if name != partition_name:
                in_names.append(name)
        elif alloc.kind == "ExternalOutput":
            out_names.append(name)
            out_avals.append(jax.core.ShapedArray(
                tuple(alloc.tensor_shape), mybir.dt.np(alloc.dtype)))
    n_params = len(in_names)
    all_in_names = list(in_names) + list(out_names)
    if partition_name is not None:
        all_in_names.append(partition_name)

    def _body(*args):
        operands = list(args)
        if partition_name is not None:
            operands.append(partition_id_tensor())
        outs = _bass_exec_p.bind(
            *operands,
            out_avals=tuple(out_avals),
            in_names=tuple(all_in_names),
            out_names=tuple(out_names),
            lowering_input_output_aliases=(),
            sim_require_finite=True,
            sim_require_nnan=True,
            nc=nc,
        )
        return tuple(outs)

    devices = jax.devices()[:_NCORES]
    mesh = Mesh(np.asarray(devices), ("core",))
    in_specs = (PartitionSpec("core"),) * (n_params + len(out_names))
    out_specs = (PartitionSpec("core"),) * len(out_names)
    fn = jax.jit(
        shard_map(_body, mesh=mesh, in_specs=in_specs,
                  out_specs=out_specs, check_rep=False),
        keep_unused=True,
    )
    _cache["k"] = (fn, in_names, out_names, out_avals, mesh)
    return _cache["k"]


def _prep_core_inputs(x, weight, indices):
    """Host-side shard prep (pure elementwise repacking, no reordering)."""
    xrows = np.ascontiguousarray(
        x.reshape(-1).astype(np.float32).reshape(_ROWS, _ES))
    idx = indices.reshape(-1)
    w = weight.reshape(-1).astype(np.float32)
    rows_all = (idx >> 6).astype(np.int16)
    offs_all = (idx & 63).astype(np.float32)
    iota = np.tile(np.arange(_ES, dtype=np.float32), (_P, 1))

    per_core = []
    for c in range(_NCORES):
        sl = slice(c * _EPC, (c + 1) * _EPC)
        # dma_gather wrapped layout: element j of a tile ->
        # idx tile [j % 16, j // 16]; replicate into partitions 16..31.
        r = rows_all[sl].reshape(_T, _N // 16, 16).transpose(0, 2, 1)
        rows32 = np.concatenate([r, r], axis=1)  # [T, 32, N/16]
        # element i of a tile -> [i % 128, i // 128]
        off_t = offs_all[sl].reshape(_T, _K, _P).transpose(0, 2, 1)
        w_t = w[sl].reshape(_T, _K, _P).transpose(0, 2, 1)
        offw = np.concatenate([off_t, w_t], axis=2)  # [T, P, 2K]
        per_core.append({
            "xrows": xrows,
            "rows": np.ascontiguousarray(rows32),
            "offw": np.ascontiguousarray(offw),
            "iota": iota,
        })
    return per_core


def kernel(x, weight, bias, indices):
    import jax
    from jax.sharding import NamedSharding, PartitionSpec

    fn, in_names, out_names, out_avals, mesh = _get_kernel()
    per_core = _prep_core_inputs(np.asarray(x), np.asarray(weight),
                                 np.asarray(indices))
    concat = []
    for name in in_names:
        concat.append(np.concatenate(
            [np.asarray(m[name]) for m in per_core], axis=0))
    zero_outs = [np.zeros((_NCORES * a.shape[0], *a.shape[1:]), a.dtype)
                 for a in out_avals]
    sharding = NamedSharding(mesh, PartitionSpec("core"))
    dargs = [jax.device_put(a, sharding) for a in concat + zero_outs]
    outs = fn(*dargs)
    jax.block_until_ready(outs)

    vo = np.asarray(outs[out_names.index("vout")]).reshape(_NCORES, _VPC)
    vals = vo.reshape(-1)  # voxel-ordered partial sums
    out = vals * _SCALE + np.asarray(bias).reshape(-1).astype(np.float32)
    out = out.reshape(256, 256)[::-1, ::-1]
    return np.ascontiguousarray(out).reshape(1, 1, 256, 256).astype(np.float32)



# revision 7
# speedup vs baseline: 2.1747x; 2.1747x over previous
"""BackProjNet kernel for 8 Trainium2 NeuronCores.

Strategy (sharding_hint: shard voxel axis):
  - 65536 voxels split across 8 cores (8192 voxels / 8.39M elements each).
  - Per element we need x[idx]*w summed per voxel (segments of 1024).
  - The sinogram is viewed as 5888 rows x 64 f32 (256B rows). For each
    element, dma_gather fetches the 256B row containing x[idx] from HBM
    into SBUF; a DVE one-hot select (off = idx & 63) picks the target
    value, multiplies by w, and reduces; the PE sums partitions per voxel.
  - Scale/bias/flip epilogue on host (O(output) work only).
"""
import sys

sys.path.insert(0, "/opt/trn_rl_repo")

import numpy as np

_P = 128
_ROWS = 5888
_ES = 64                 # f32 per gathered row (256B)
_N = 8192                # elements per tile
_K = _N // _P            # 64 free slots per partition per tile
_NCORES = 8
_VOX = 65536
_VPC = _VOX // _NCORES   # 8192 voxels per core
_EPC = _VPC * 1024       # 8388608 elements per core
_T = _EPC // _N          # 1024 tiles per core
_VPT = _N // 1024        # 8 voxels per tile
_SCALE = np.float32(2.0 * np.pi / (2.0 * 512 * 2))

_cache = {}


def _patch_tile_drain():
    import bass_rust
    import concourse.tile as tile_mod
    from concourse.vector_clock import ScopedClock

    if getattr(tile_mod.TileContext, "_drain_waits_split", False):
        return

    def _drain_and_barrier(self, tick_clock, wait_clock):
        nc = self.nc
        probe = nc.sync.nop()
        wait_clock.add_sem_waits(
            probe.ins, ScopedClock({None: tick_clock.global_clock})
        )
        si = probe.ins.sync_info
        waits = list(si.on_wait) if si is not None else []
        if len(waits) > 1:
            si.on_wait = [waits[0]]
            for w in waits[1:]:
                extra = nc.sync.nop()
                if extra.ins.sync_info is None:
                    extra.ins.sync_info = bass_rust.SyncInfo(
                        on_wait=[w], on_update=[]
                    )
                else:
                    extra.ins.sync_info.on_wait = [w]
        nc.sync.drain()
        nc.all_engine_barrier()
        assert self.sems is not None
        popped = nc._tile_sem_poison_stack.pop()
        assert popped is self._sem_poison
        nc.clear_and_free_semaphores(list(self.sems.allocated().values()))
        nc.all_engine_barrier()

    tile_mod.TileContext._drain_and_barrier = _drain_and_barrier
    tile_mod.TileContext._drain_waits_split = True


def _split_multi_waits(nc, max_waits=1):
    from concourse import mybir

    ctr = 0
    for f in nc.m.functions:
        for bb in f.blocks:
            insts = list(bb.instructions)
            changed = False
            out = []
            for inst in insts:
                si = getattr(inst, "sync_info", None)
                if si is not None and len(si.on_wait) > max_waits:
                    waits = list(si.on_wait)
                    for w in waits[max_waits:]:
                        ctr += 1
                        nop = mybir.InstNoOp(
                            name=f"wsplit-{ctr}",
                            sync_info=mybir.SyncInfo(on_wait=[w], on_update=[]),
                            bass_nofuse=True,
                            engine=inst.engine,
                        )
                        out.append(nop)
                    si.on_wait = waits[:max_waits]
                    changed = True
                out.append(inst)
            if changed:
                try:
                    bb.instructions = out
                except Exception:
                    bb.clear_instructions()
                    for i in out:
                        bb.add_instruction(i)
    return ctr


def _build_nc():
    from contextlib import ExitStack

    from concourse import bass, library_config, mybir
    from concourse.tile import TileContext

    nc = bass.Bass(
        "TRN2",
        num_devices=_NCORES,
        dynamic_dma_scratch_size=65536,
        num_swdge_queues=4,
    )
    xrows = nc.dram_tensor("xrows", [_ROWS, _ES], mybir.dt.float32,
                           kind="ExternalInput")
    rows = nc.dram_tensor("rows", [_T, 128, _N // 16], mybir.dt.int16,
                          kind="ExternalInput")
    offw = nc.dram_tensor("offw", [_T, _P, 2 * _K], mybir.dt.float32,
                          kind="ExternalInput")
    iota = nc.dram_tensor("iota", [_P, _ES], mybir.dt.float32,
                          kind="ExternalInput")
    vout = nc.dram_tensor("vout", [1, _VPC], mybir.dt.float32,
                          kind="ExternalOutput")

    with TileContext(nc) as tc, ExitStack() as ctx:
        nc.gpsimd.load_library(library_config.mlp)
        nreg = nc.gpsimd.to_reg(_N)

        cpool = ctx.enter_context(tc.tile_pool(name="const", bufs=1))
        ipool = ctx.enter_context(tc.tile_pool(name="idx", bufs=3))
        spool = ctx.enter_context(tc.tile_pool(name="stream", bufs=3))
        gpool = ctx.enter_context(tc.tile_pool(name="g", bufs=3))
        epool = ctx.enter_context(tc.tile_pool(name="eq", bufs=3))
        rpool = ctx.enter_context(tc.tile_pool(name="red", bufs=3))
        opool = ctx.enter_context(tc.tile_pool(name="out", bufs=1))
        pspool = ctx.enter_context(
            tc.tile_pool(name="ps", bufs=2, space="PSUM"))

        tio = cpool.tile([_P, _ES], mybir.dt.float32)
        nc.sync.dma_start(tio[:], iota[:, :])
        ones = cpool.tile([_P, 1], mybir.dt.float32)
        nc.vector.memset(ones[:], 1.0)
        voxrow = opool.tile([1, _VPC], mybir.dt.float32)

        for t in range(_T):
            it = ipool.tile([_P, _N // 16], mybir.dt.int16, tag="it")
            nc.sync.dma_start(it[:], rows[t, :, :])
            st = spool.tile([_P, 2 * _K], mybir.dt.float32, tag="st")
            nc.sync.dma_start(st[:], offw[t, :, :])

            g = gpool.tile([_P, _K * _ES], mybir.dt.float32, tag="g")
            g3 = bass.AP(g.tensor, g.offset,
                         [g.ap[0], [_ES, _K], [1, _ES]])
            nc.gpsimd.dma_gather(
                g3, xrows[:, :], it[:], num_idxs=_N, num_idxs_reg=nreg,
                elem_size=_ES, single_packet=False, queue_num=t % 4)

            # EQ[p, k, o] = (off[p, k] == o)
            eq = epool.tile([_P, _K * _ES], mybir.dt.float32, tag="eq")
            eq3 = bass.AP(eq.tensor, eq.offset,
                          [eq.ap[0], [_ES, _K], [1, _ES]])
            off_sl = st[:, 0:_K]
            off_b = bass.AP(off_sl.tensor, off_sl.offset,
                            [off_sl.ap[0], [1, _K], [0, _ES]])
            io_b = bass.AP(tio.tensor, tio.offset,
                           [tio.ap[0], [0, _K], [1, _ES]])
            nc.vector.tensor_tensor(eq3, off_b, io_b,
                                    mybir.AluOpType.is_equal)
            # EQ *= w (broadcast over o)
            w_sl = st[:, _K:2 * _K]
            w_b = bass.AP(w_sl.tensor, w_sl.offset,
                          [w_sl.ap[0], [1, _K], [0, _ES]])
            nc.vector.tensor_tensor(eq3, eq3, w_b, mybir.AluOpType.mult)
            # EQ *= gathered
            nc.vector.tensor_tensor(eq3, eq3, g3, mybir.AluOpType.mult)
            # reduce over o: [P, K]
            r1 = rpool.tile([_P, _K], mybir.dt.float32, tag="r1")
            nc.vector.tensor_reduce(
                bass.AP(r1.tensor, r1.offset, [r1.ap[0], [1, _K], [1, 1]]),
                eq3, mybir.AxisListType.X, mybir.AluOpType.add)
            # reduce k-groups of 8: [P, VPT]
            r2 = rpool.tile([_P, _VPT], mybir.dt.float32, tag="r2")
            r1v = bass.AP(r1.tensor, r1.offset,
                          [r1.ap[0], [_K // _VPT, _VPT], [1, _K // _VPT]])
            r2v = bass.AP(r2.tensor, r2.offset,
                          [r2.ap[0], [1, _VPT], [1, 1]])
            nc.vector.tensor_reduce(r2v, r1v, mybir.AxisListType.X,
                                    mybir.AluOpType.add)
            # partition sum via PE: [1, VPT]
            ps = pspool.tile([1, _VPT], mybir.dt.float32, tag="ps")
            nc.tensor.matmul(ps[:], ones[:], r2[:], start=True, stop=True)
            nc.vector.tensor_copy(voxrow[0:1, t * _VPT:(t + 1) * _VPT],
                                  ps[:])

        nc.sync.dma_start(vout[:, :], voxrow[:])
    return nc


def _get_kernel():
    if "k" in _cache:
        return _cache["k"]
    _patch_tile_drain()
    from concourse import mybir
    from concourse.bass2jax import install_neuronx_cc_hook, partition_id_tensor, _bass_exec_p
    import jax
    from jax.sharding import Mesh, PartitionSpec
    from jax.experimental.shard_map import shard_map

    install_neuronx_cc_hook()
    nc = _build_nc()
    mybir.codegen_inst_isa_subclasses(nc)
    _split_multi_waits(nc)

    partition_name = (nc.partition_id_tensor.name
                      if nc.partition_id_tensor else None)
    in_names, out_names, out_avals = [], [], []
    for alloc in nc.m.functions[0].allocations:
        if not isinstance(alloc, mybir.MemoryLocationSet):
            continue
        name = alloc.memorylocations[0].name
        if alloc.kind == "ExternalInput":
            if name != partition_name:
                in_names.append(name)
        elif alloc.kind == "ExternalOutput":
            out_names.append(name)
            out_avals.append(jax.core.ShapedArray(
                tuple(alloc.tensor_shape), mybir.dt.np(alloc.dtype)))
    n_params = len(in_names)
    all_in_names = list(in_names) + list(out_names)
    if partition_name is not None:
        all_in_names.append(partition_name)

    def _body(*args):
        operands = list(args)
        if partition_name is not None:
            operands.append(partition_id_tensor())
        outs = _bass_exec_p.bind(
            *operands,
            out_avals=tuple(out_avals),
            in_names=tuple(all_in_names),
            out_names=tuple(out_names),
            lowering_input_output_aliases=(),
            sim_require_finite=True,
            sim_require_nnan=True,
            nc=nc,
        )
        return tuple(outs)

    devices = jax.devices()[:_NCORES]
    mesh = Mesh(np.asarray(devices), ("core",))
    in_specs = (PartitionSpec("core"),) * (n_params + len(out_names))
    out_specs = (PartitionSpec("core"),) * len(out_names)
    fn = jax.jit(
        shard_map(_body, mesh=mesh, in_specs=in_specs,
                  out_specs=out_specs, check_rep=False),
        keep_unused=True,
    )
    _cache["k"] = (fn, in_names, out_names, out_avals, mesh)
    return _cache["k"]


def _prep_core_inputs(x, weight, indices):
    """Host-side shard prep (pure elementwise repacking, no reordering)."""
    xrows = np.ascontiguousarray(
        x.reshape(-1).astype(np.float32).reshape(_ROWS, _ES))
    idx = indices.reshape(-1)
    w = weight.reshape(-1).astype(np.float32)
    rows_all = (idx >> 6).astype(np.int16)
    offs_all = (idx & 63).astype(np.float32)
    iota = np.tile(np.arange(_ES, dtype=np.float32), (_P, 1))

    per_core = []
    for c in range(_NCORES):
        sl = slice(c * _EPC, (c + 1) * _EPC)
        # dma_gather wrapped layout: element j of a tile ->
        # idx tile [j % 16, j // 16]; replicate into partitions 16..31.
        r = rows_all[sl].reshape(_T, _N // 16, 16).transpose(0, 2, 1)
        rows128 = np.tile(r, (1, 8, 1))  # [T, 128, N/16] (one copy per queue pair)
        # element i of a tile -> [i % 128, i // 128]
        off_t = offs_all[sl].reshape(_T, _K, _P).transpose(0, 2, 1)
        w_t = w[sl].reshape(_T, _K, _P).transpose(0, 2, 1)
        offw = np.concatenate([off_t, w_t], axis=2)  # [T, P, 2K]
        per_core.append({
            "xrows": xrows,
            "rows": np.ascontiguousarray(rows128),
            "offw": np.ascontiguousarray(offw),
            "iota": iota,
        })
    return per_core


def kernel(x, weight, bias, indices):
    import jax
    from jax.sharding import NamedSharding, PartitionSpec

    fn, in_names, out_names, out_avals, mesh = _get_kernel()
    per_core = _prep_core_inputs(np.asarray(x), np.asarray(weight),
                                 np.asarray(indices))
    concat = []
    for name in in_names:
        concat.append(np.concatenate(
            [np.asarray(m[name]) for m in per_core], axis=0))
    zero_outs = [np.zeros((_NCORES * a.shape[0], *a.shape[1:]), a.dtype)
                 for a in out_avals]
    sharding = NamedSharding(mesh, PartitionSpec("core"))
    dargs = [jax.device_put(a, sharding) for a in concat + zero_outs]
    outs = fn(*dargs)
    jax.block_until_ready(outs)

    vo = np.asarray(outs[out_names.index("vout")]).reshape(_NCORES, _VPC)
    vals = vo.reshape(-1)  # voxel-ordered partial sums
    out = vals * _SCALE + np.asarray(bias).reshape(-1).astype(np.float32)
    out = out.reshape(256, 256)[::-1, ::-1]
    return np.ascontiguousarray(out).reshape(1, 1, 256, 256).astype(np.float32)



# revision 13
# speedup vs baseline: 2.6368x; 1.2125x over previous
"""BackProjNet kernel for 8 Trainium2 NeuronCores.

Strategy (sharding_hint: shard voxel axis):
  - 65536 voxels split across 8 cores (8192 voxels / 8.39M elements each).
  - Per element we need x[idx]*w summed per voxel (segments of 1024).
  - The sinogram is viewed as 5888 rows x 64 f32 (256B rows). For each
    element, dma_gather fetches the 256B row containing x[idx] from HBM
    into SBUF; a DVE one-hot select (off = idx & 63) picks the target
    value, multiplies by w, and reduces; the PE sums partitions per voxel.
  - Scale/bias/flip epilogue on host (O(output) work only).
"""
import sys

sys.path.insert(0, "/opt/trn_rl_repo")

import numpy as np

_P = 128
_ROWS = 5888
_ES = 64                 # f32 per gathered row (256B)
_N = 8192                # elements per tile
_K = _N // _P            # 64 free slots per partition per tile
_NCORES = 8
_VOX = 65536
_VPC = _VOX // _NCORES   # 8192 voxels per core
_EPC = _VPC * 1024       # 8388608 elements per core
_T = _EPC // _N          # 1024 tiles per core
_VPT = _N // 1024        # 8 voxels per tile
_SCALE = np.float32(2.0 * np.pi / (2.0 * 512 * 2))

_cache = {}


def _patch_tile_drain():
    import bass_rust
    import concourse.tile as tile_mod
    from concourse.vector_clock import ScopedClock

    if getattr(tile_mod.TileContext, "_drain_waits_split", False):
        return

    def _drain_and_barrier(self, tick_clock, wait_clock):
        nc = self.nc
        probe = nc.sync.nop()
        wait_clock.add_sem_waits(
            probe.ins, ScopedClock({None: tick_clock.global_clock})
        )
        si = probe.ins.sync_info
        waits = list(si.on_wait) if si is not None else []
        if len(waits) > 1:
            si.on_wait = [waits[0]]
            for w in waits[1:]:
                extra = nc.sync.nop()
                if extra.ins.sync_info is None:
                    extra.ins.sync_info = bass_rust.SyncInfo(
                        on_wait=[w], on_update=[]
                    )
                else:
                    extra.ins.sync_info.on_wait = [w]
        nc.sync.drain()
        nc.all_engine_barrier()
        assert self.sems is not None
        popped = nc._tile_sem_poison_stack.pop()
        assert popped is self._sem_poison
        nc.clear_and_free_semaphores(list(self.sems.allocated().values()))
        nc.all_engine_barrier()

    tile_mod.TileContext._drain_and_barrier = _drain_and_barrier
    tile_mod.TileContext._drain_waits_split = True


def _split_multi_waits(nc, max_waits=1):
    from concourse import mybir

    ctr = 0
    for f in nc.m.functions:
        for bb in f.blocks:
            insts = list(bb.instructions)
            changed = False
            out = []
            for inst in insts:
                si = getattr(inst, "sync_info", None)
                if si is not None and len(si.on_wait) > max_waits:
                    waits = list(si.on_wait)
                    for w in waits[max_waits:]:
                        ctr += 1
                        nop = mybir.InstNoOp(
                            name=f"wsplit-{ctr}",
                            sync_info=mybir.SyncInfo(on_wait=[w], on_update=[]),
                            bass_nofuse=True,
                            engine=inst.engine,
                        )
                        out.append(nop)
                    si.on_wait = waits[:max_waits]
                    changed = True
                out.append(inst)
            if changed:
                try:
                    bb.instructions = out
                except Exception:
                    bb.clear_instructions()
                    for i in out:
                        bb.add_instruction(i)
    return ctr


def _build_nc():
    from contextlib import ExitStack

    from concourse import bass, library_config, mybir
    from concourse.tile import TileContext

    nc = bass.Bass(
        "TRN2",
        num_devices=_NCORES,
        dynamic_dma_scratch_size=65536,
        num_swdge_queues=4,
    )
    xrows = nc.dram_tensor("xrows", [_ROWS, _ES], mybir.dt.float32,
                           kind="ExternalInput")
    rows = nc.dram_tensor("rows", [_T, 128, _N // 16], mybir.dt.int16,
                          kind="ExternalInput")
    offw = nc.dram_tensor("offw", [_T, _P, 2 * _K], mybir.dt.float32,
                          kind="ExternalInput")
    iota = nc.dram_tensor("iota", [_P, _ES], mybir.dt.float32,
                          kind="ExternalInput")
    vout = nc.dram_tensor("vout", [_T, _VPT], mybir.dt.float32,
                          kind="ExternalOutput")

    with TileContext(nc) as tc, ExitStack() as ctx:
        nc.gpsimd.load_library(library_config.mlp)
        nreg = nc.gpsimd.to_reg(_N)

        cpool = ctx.enter_context(tc.tile_pool(name="const", bufs=1))
        ipool = ctx.enter_context(tc.tile_pool(name="idx", bufs=6))
        spool = ctx.enter_context(tc.tile_pool(name="stream", bufs=6))
        gpool = ctx.enter_context(tc.tile_pool(name="g", bufs=6))
        epool = ctx.enter_context(tc.tile_pool(name="eq", bufs=3))
        rpool = ctx.enter_context(tc.tile_pool(name="red", bufs=4))
        pspool = ctx.enter_context(
            tc.tile_pool(name="ps", bufs=4, space="PSUM"))

        tio = cpool.tile([_P, _ES], mybir.dt.float32)
        nc.sync.dma_start(tio[:], iota[:, :])
        ones = cpool.tile([_P, 1], mybir.dt.float32)
        nc.vector.memset(ones[:], 1.0)

        for t in range(_T):
            it = ipool.tile([_P, _N // 16], mybir.dt.int16, tag="it")
            nc.sync.dma_start(it[:], rows[t, :, :])
            st = spool.tile([_P, 2 * _K], mybir.dt.float32, tag="st")
            nc.sync.dma_start(st[:], offw[t, :, :])

            g = gpool.tile([_P, _K * _ES], mybir.dt.float32, tag="g")
            g3 = bass.AP(g.tensor, g.offset,
                         [g.ap[0], [_ES, _K], [1, _ES]])
            nc.gpsimd.dma_gather(
                g3, xrows[:, :], it[:], num_idxs=_N, num_idxs_reg=nreg,
                elem_size=_ES, single_packet=False, queue_num=t % 4)

            # EQ[p, k, o] = (off[p, k] == o)
            eq = epool.tile([_P, _K * _ES], mybir.dt.float32, tag="eq")
            eq3 = bass.AP(eq.tensor, eq.offset,
                          [eq.ap[0], [_ES, _K], [1, _ES]])
            off_sl = st[:, 0:_K]
            off_b = bass.AP(off_sl.tensor, off_sl.offset,
                            [off_sl.ap[0], [1, _K], [0, _ES]])
            io_b = bass.AP(tio.tensor, tio.offset,
                           [tio.ap[0], [0, _K], [1, _ES]])
            nc.vector.tensor_tensor(eq3, off_b, io_b,
                                    mybir.AluOpType.is_equal)
            # EQ *= w (broadcast over o)
            w_sl = st[:, _K:2 * _K]
            w_b = bass.AP(w_sl.tensor, w_sl.offset,
                          [w_sl.ap[0], [1, _K], [0, _ES]])
            nc.vector.tensor_tensor(eq3, eq3, w_b, mybir.AluOpType.mult)
            # EQ *= gathered
            nc.vector.tensor_tensor(eq3, eq3, g3, mybir.AluOpType.mult)
            # reduce over o: [P, K]
            r1 = rpool.tile([_P, _K], mybir.dt.float32, tag="r1")
            nc.vector.tensor_reduce(
                bass.AP(r1.tensor, r1.offset, [r1.ap[0], [1, _K], [1, 1]]),
                eq3, mybir.AxisListType.X, mybir.AluOpType.add)
            # reduce k-groups of 8: [P, VPT]
            r2 = rpool.tile([_P, _VPT], mybir.dt.float32, tag="r2")
            r1v = bass.AP(r1.tensor, r1.offset,
                          [r1.ap[0], [_K // _VPT, _VPT], [1, _K // _VPT]])
            r2v = bass.AP(r2.tensor, r2.offset,
                          [r2.ap[0], [1, _VPT], [1, 1]])
            nc.vector.tensor_reduce(r2v, r1v, mybir.AxisListType.X,
                                    mybir.AluOpType.add)
            # partition sum via PE: [1, VPT]
            ps = pspool.tile([1, _VPT], mybir.dt.float32, tag="ps")
            nc.tensor.matmul(ps[:], ones[:], r2[:], start=True, stop=True)
            sb = rpool.tile([1, _VPT], mybir.dt.float32, tag="sb")
            nc.vector.tensor_copy(sb[:], ps[:])
            nc.sync.dma_start(vout[t, :], sb[:])
    return nc


def _get_kernel():
    if "k" in _cache:
        return _cache["k"]
    _patch_tile_drain()
    from concourse import mybir
    from concourse.bass2jax import install_neuronx_cc_hook, partition_id_tensor, _bass_exec_p
    import jax
    from jax.sharding import Mesh, PartitionSpec
    from jax.experimental.shard_map import shard_map

    install_neuronx_cc_hook()
    nc = _build_nc()
    mybir.codegen_inst_isa_subclasses(nc)
    _split_multi_waits(nc)

    partition_name = (nc.partition_id_tensor.name
                      if nc.partition_id_tensor else None)
    in_names, out_names, out_avals = [], [], []
    for alloc in nc.m.functions[0].allocations:
        if not isinstance(alloc, mybir.MemoryLocationSet):
            continue
        name = alloc.memorylocations[0].name
        if alloc.kind == "ExternalInput":
            if name != partition_name:
                in_names.append(name)
        elif alloc.kind == "ExternalOutput":
            out_names.append(name)
            out_avals.append(jax.core.ShapedArray(
                tuple(alloc.tensor_shape), mybir.dt.np(alloc.dtype)))
    n_params = len(in_names)
    all_in_names = list(in_names) + list(out_names)
    if partition_name is not None:
        all_in_names.append(partition_name)

    def _body(*args):
        operands = list(args)
        if partition_name is not None:
            operands.append(partition_id_tensor())
        outs = _bass_exec_p.bind(
            *operands,
            out_avals=tuple(out_avals),
            in_names=tuple(all_in_names),
            out_names=tuple(out_names),
            lowering_input_output_aliases=(),
            sim_require_finite=True,
            sim_require_nnan=True,
            nc=nc,
        )
        return tuple(outs)

    devices = jax.devices()[:_NCORES]
    mesh = Mesh(np.asarray(devices), ("core",))
    in_specs = (PartitionSpec("core"),) * (n_params + len(out_names))
    out_specs = (PartitionSpec("core"),) * len(out_names)
    fn = jax.jit(
        shard_map(_body, mesh=mesh, in_specs=in_specs,
                  out_specs=out_specs, check_rep=False),
        keep_unused=True,
    )
    _cache["k"] = (fn, in_names, out_names, out_avals, mesh)
    return _cache["k"]


def _prep_core_inputs(x, weight, indices):
    """Host-side shard prep (pure elementwise repacking, no reordering)."""
    xrows = np.ascontiguousarray(
        x.reshape(-1).astype(np.float32).reshape(_ROWS, _ES))
    idx = indices.reshape(-1)
    w = weight.reshape(-1).astype(np.float32)
    rows_all = (idx >> 6).astype(np.int16)
    offs_all = (idx & 63).astype(np.float32)
    iota = np.tile(np.arange(_ES, dtype=np.float32), (_P, 1))

    per_core = []
    for c in range(_NCORES):
        sl = slice(c * _EPC, (c + 1) * _EPC)
        # dma_gather wrapped layout: element j of a tile ->
        # idx tile [j % 16, j // 16]; replicate into partitions 16..31.
        r = rows_all[sl].reshape(_T, _N // 16, 16).transpose(0, 2, 1)
        rows128 = np.tile(r, (1, 8, 1))  # [T, 128, N/16] (one copy per queue pair)
        # element i of a tile -> [i % 128, i // 128]
        off_t = offs_all[sl].reshape(_T, _K, _P).transpose(0, 2, 1)
        w_t = w[sl].reshape(_T, _K, _P).transpose(0, 2, 1)
        offw = np.concatenate([off_t, w_t], axis=2)  # [T, P, 2K]
        per_core.append({
            "xrows": xrows,
            "rows": np.ascontiguousarray(rows128),
            "offw": np.ascontiguousarray(offw),
            "iota": iota,
        })
    return per_core


def kernel(x, weight, bias, indices):
    import jax
    from jax.sharding import NamedSharding, PartitionSpec

    fn, in_names, out_names, out_avals, mesh = _get_kernel()
    per_core = _prep_core_inputs(np.asarray(x), np.asarray(weight),
                                 np.asarray(indices))
    concat = []
    for name in in_names:
        concat.append(np.concatenate(
            [np.asarray(m[name]) for m in per_core], axis=0))
    zero_outs = [np.zeros((_NCORES * a.shape[0], *a.shape[1:]), a.dtype)
                 for a in out_avals]
    sharding = NamedSharding(mesh, PartitionSpec("core"))
    dargs = [jax.device_put(a, sharding) for a in concat + zero_outs]
    outs = fn(*dargs)
    jax.block_until_ready(outs)

    vo = np.asarray(outs[out_names.index("vout")]).reshape(_NCORES, _VPC)
    vals = vo.reshape(-1)  # voxel-ordered partial sums
    out = vals * _SCALE + np.asarray(bias).reshape(-1).astype(np.float32)
    out = out.reshape(256, 256)[::-1, ::-1]
    return np.ascontiguousarray(out).reshape(1, 1, 256, 256).astype(np.float32)

